# revision 30
# baseline (speedup 1.0000x reference)
"""Causal self-attention kernel for Trainium2, 8 NeuronCores.

Problem: B=2, S=2048, D=1024, H=16 heads, Hd=64. fp32.
  q/k/v = x @ W{q,k,v}.T + b;  att = softmax(causal(q k^T / 8));  y = att v
  out = y @ Wp.T + bp

Sharding (per spec hint, batch x head-group): core c -> batch b=c//4,
head-group g=c%4 (4 heads = 256 of 1024 dims). Each core computes its
QKV slice on its batch, causal attention for its 4 heads, and a partial
output projection out_c = y_c @ Wp[:, g-slice].T (row-parallel TP).
Host unshard: out[b] = sum_g out_partial[4b+g] + (bv @ Wp.T + bp).

Exact algebraic folds used (softmax-invariant / row-sum-1):
  - bk dropped: adds a per-query-row constant to scores -> softmax invariant.
  - bv folded to host: softmax rows sum to 1, so P@(v+bv) = P@v + bv; the bv
    contribution to out is the constant row bv @ Wp.T, added on host.

On-device layout: everything transposed ("S.T layout", [k-part, q-free]) so
causal softmax normalization is per-column, P.T feeds P@V directly as the
moving operand (no PE transposes anywhere), and V carries 64 ones-columns
per head so the matmul emits broadcast softmax denominators for free.
Matmuls run in float32r (fp32 with 11-bit mantissa, 4x PE throughput);
inputs are pre-rounded on host, on-chip tensors are rounded by the engine
writing them (dtype float32r on the SBUF tile).
"""
import json
import sys

sys.path.insert(0, "/opt/trn_rl_repo")

import numpy as np

import concourse.bass as bass
import concourse.mybir as mybir
import concourse.tile as tile
from concourse.bass_utils import run_bass_kernel_spmd

F32 = mybir.dt.float32
F32R = mybir.dt.float32r
AF = mybir.ActivationFunctionType
OP = mybir.AluOpType

S = 2048          # tokens per batch (= per core)
D = 1024          # model dim
HL = 4            # heads per core
HD = 64           # head dim
DL = HL * HD      # local dims per core (256)
MASKVAL = -1e30


# ---------------------------------------------------------------------------
# Wait-legalization: this walrus enforces <=1 sem-wait per instruction
# (<=2 for EventSemaphore); Tile's wait-assignment can attach more. Spill
# extras onto EventSemaphore instructions inserted before the offender.
def _legalize_waits_json(bir_bytes: bytes) -> bytes:
    j = json.loads(bir_bytes)
    for fn in j["functions"]:
        for bb in fn["blocks"]:
            out = []
            for inst in bb["instructions"]:
                si = inst.get("sync_info") or {}
                ws = si.get("on_wait") or []
                cap = 2 if inst.get("opcode") == "EventSemaphore" else 1
                if len(ws) > cap:
                    extras, keep = ws[:-cap], ws[-cap:]
                    k = 0
                    while extras:
                        chunk, extras = extras[:2], extras[2:]
                        out.append({
                            "debug": inst.get("debug", 0),
                            "engine": inst["engine"],
                            "ins": [],
                            "name": f"{inst['name']}_wfix{k}",
                            "opcode": "EventSemaphore",
                            "outs": [],
                            "sync_info": {"on_update": [], "on_wait": chunk},
                        })
                        k += 1
                    si["on_wait"] = keep
                out.append(inst)
            bb["instructions"] = out
    return json.dumps(j).encode()


def _install_legalizer(nc):
    orig = nc.to_json_bytes
    nc.to_json_bytes = lambda: _legalize_waits_json(orig())


def _round_fp32r(a: np.ndarray) -> np.ndarray:
    """Round fp32 to fp32r (11-bit mantissa, round-half-up) like the HW cast."""
    bits = np.ascontiguousarray(a, dtype=np.float32).view(np.uint32)
    return ((bits + 0x800) & 0xFFFFF000).view(np.float32)


def build_nc() -> bass.Bass:
    nc = bass.Bass(trn_type="TRN2", num_devices=8)

    xT = nc.dram_tensor("xT", [D, S], F32R, kind="ExternalInput")       # x[b].T
    wq = nc.dram_tensor("wq", [D, DL], F32R, kind="ExternalInput")      # Wq_g.T
    wk = nc.dram_tensor("wk", [D, DL], F32R, kind="ExternalInput")      # Wk_g.T
    wv = nc.dram_tensor("wv", [D, DL], F32R, kind="ExternalInput")      # Wv_g.T
    wp = nc.dram_tensor("wp", [DL, D], F32R, kind="ExternalInput")      # Wp[:,sl].T
    bq = nc.dram_tensor("bq", [DL], F32, kind="ExternalInput")
    mask = nc.dram_tensor("mask", [128, 128], F32, kind="ExternalInput")
    out = nc.dram_tensor("out", [S, D], F32, kind="ExternalOutput")

    with tile.TileContext(nc) as tc:
        with tc.tile_pool(name="const", bufs=1) as const, \
             tc.tile_pool(name="acts", bufs=1) as acts, \
             tc.tile_pool(name="xin", bufs=2) as xpool, \
             tc.tile_pool(name="pt", bufs=4) as ptp, \
             tc.tile_pool(name="rc", bufs=1) as rcp, \
             tc.tile_pool(name="outp", bufs=3) as outp, \
             tc.tile_pool(name="ps", bufs=1, space="PSUM") as ps:
            # --- weight/const tiles; DMAs split per k-chunk so the first
            # matmuls unblock after ~2 small DMAs, not the full preload ---
            wq_sb = const.tile([128, 8, DL], F32R)
            wk_sb = const.tile([128, 8, DL], F32R)
            wv_sb = const.tile([128, 8, DL], F32R)
            wp_sb = const.tile([128, 2, D], F32R)
            bq_sb = const.tile([128, 2], F32)
            mask_sb = const.tile([128, 128], F32)
            # wide mask for n=128 chunks widened to n=256 (fp32r matmuls with
            # free-dim < 256 run at 1/4 rate): cols 0:128 fully masked,
            # cols 128:256 the causal triangle
            maskw_sb = const.tile([128, 256], F32)

            # persistent activations
            qT_sb = [acts.tile([128, S], F32R, name=f"qT{m}") for m in range(2)]
            kT_sb = [acts.tile([128, S], F32R, name=f"kT{m}") for m in range(2)]
            yT_sb = [acts.tile([128, S], F32R, name=f"yT{m}") for m in range(2)]
            # v with interleaved ones-columns: head h at cols [128h,128h+64) = v,
            # [128h+64,128h+128) = 1.0 -> P@V emits broadcast denominators in
            # psum rows 64:128
            vO_sb = [acts.tile([128, 4 * 128], F32R, name=f"vO{i}") for i in range(16)]

            x3 = xT[:].rearrange("(kc p) t -> p kc t", p=128)
            wq3 = wq[:].rearrange("(kc p) m -> p kc m", p=128)
            wk3 = wk[:].rearrange("(kc p) m -> p kc m", p=128)
            wv3 = wv[:].rearrange("(kc p) m -> p kc m", p=128)

            xsb_tiles = {}

            def dma_x(t4):
                xsb = xpool.tile([128, 8, 512], F32R, tag="x", name=f"x{t4}")
                xsb_tiles[t4] = xsb
                for half in range(2):
                    nc.sync.dma_start(
                        out=xsb[:, 4 * half:4 * half + 4, :],
                        in_=x3[:, 4 * half:4 * half + 4, 512 * t4:512 * t4 + 512])
                return xsb

            # startup order: the prologue only touches the m=0 halves of
            # wq/wk (heads 0-1); stream x0[kc]+wq_m0[kc] in kc order quartered
            # so the first matmul unblocks after ~0.75 MB, and defer the m=1
            # weight halves (first consumed mid-attention(0)) past everything
            # the prologue needs
            xsb0 = xpool.tile([128, 8, 512], F32R, tag="x", name="x0")
            xsb_tiles[0] = xsb0
            m0, m1 = slice(0, 128), slice(128, 256)
            for qtr in range(4):
                sl = slice(2 * qtr, 2 * qtr + 2)
                nc.sync.dma_start(out=xsb0[:, sl, :], in_=x3[:, sl, 0:512])
                nc.sync.dma_start(out=wq_sb[:, sl, m0], in_=wq3[:, sl, m0])
            h0, h1 = slice(0, 4), slice(4, 8)
            nc.sync.dma_start(out=wk_sb[:, h0, m0], in_=wk3[:, h0, m0])
            nc.sync.dma_start(out=wk_sb[:, h1, m0], in_=wk3[:, h1, m0])
            nc.sync.dma_start(out=wv_sb[:, h0, :], in_=wv3[:, h0, :])
            nc.sync.dma_start(out=wv_sb[:, h1, :], in_=wv3[:, h1, :])
            nc.sync.dma_start(out=bq_sb, in_=bq[:].rearrange("(m p) -> p m", p=128))
            nc.sync.dma_start(out=mask_sb, in_=mask[:])
            nc.vector.memset(maskw_sb[:, 0:128], MASKVAL)
            nc.vector.tensor_copy(maskw_sb[:, 128:256], mask_sb)
            nc.sync.dma_start(out=wq_sb[:, :, m1], in_=wq3[:, :, m1])
            nc.sync.dma_start(out=wk_sb[:, :, m1], in_=wk3[:, :, m1])
            for m in range(2):
                nc.sync.dma_start(out=wp_sb[:, m, :],
                                  in_=wp[:].rearrange("(m p) n -> p m n", p=128)[:, m, :])
            for i in range(16):
                v3 = vO_sb[i].rearrange("p (h c) -> p h c", h=HL)
                nc.vector.memset(v3[:, :, HD:128].bitcast(F32), 1.0)

            # --- keyed work units (one PE psum-group each), interleaved into
            # the attention stream as filler so the in-order PE never starves
            # while ACT chews exps ---
            def unit_qk(which, t4, m):
                wsb, dst = (wq_sb, qT_sb) if which == "q" else (wk_sb, kT_sb)

                def go():
                    ts512 = slice(512 * t4, 512 * t4 + 512)
                    xsb = xsb_tiles[t4]
                    dsl = slice(128 * m, 128 * m + 128)
                    p = ps.tile([128, 512], F32, tag="mm", bufs=2,
                                name=f"{which}{t4}{m}")
                    for kc in range(8):
                        nc.tensor.matmul(p[:, :], wsb[:, kc, dsl], xsb[:, kc, :],
                                         start=(kc == 0), stop=(kc == 7))
                    if which == "q":
                        nc.vector.tensor_scalar_add(dst[m][:, ts512], p,
                                                    bq_sb[:, m:m + 1])
                    else:
                        nc.vector.tensor_copy(dst[m][:, ts512], p)
                return go

            def unit_v(t4, si):
                def go():
                    xsb = xsb_tiles[t4]
                    tl = slice(128 * si, 128 * si + 128)
                    p = ps.tile([128, DL], F32, tag="mm", bufs=2, name=f"v{t4}{si}")
                    for kc in range(8):
                        nc.tensor.matmul(p[:, :], xsb[:, kc, tl], wv_sb[:, kc, :],
                                         start=(kc == 0), stop=(kc == 7))
                    v3 = vO_sb[4 * t4 + si].rearrange("p (h c) -> p h c", h=HL)
                    nc.vector.tensor_copy(
                        v3[:, :, 0:HD], p[:].rearrange("p (h c) -> p h c", h=HL))
                return go

            def unit_pj(j, sp, nsub=2):
                """Projection of nsub*128 tokens (one out DMA)."""
                def go():
                    t0 = 512 * j + 256 * sp
                    osb = outp.tile([128, 2, D], F32, tag="o", name=f"o{j}{sp}")
                    for sub in range(nsub):
                        for ncol in range(2):
                            pj = ps.tile([128, 512], F32, tag="mm", bufs=2,
                                         name=f"pj{j}{sp}{sub}{ncol}")
                            for m2 in range(2):
                                nc.tensor.matmul(
                                    pj[:, :],
                                    yT_sb[m2][:, t0 + 128 * sub:t0 + 128 * sub + 128],
                                    wp_sb[:, m2, 512 * ncol:512 * ncol + 512],
                                    start=(m2 == 0), stop=(m2 == 1))
                            nc.vector.tensor_copy(
                                osb[:, sub, 512 * ncol:512 * ncol + 512], pj)
                        nc.scalar.dma_start(
                            out=out[t0 + 128 * sub:t0 + 128 * sub + 128, :],
                            in_=osb[:, sub, :])
                return go

            # prologue: just enough QKV for attention(0, h0) to start
            unit_qk("q", 0, 0)()
            unit_qk("k", 0, 0)()
            for si in range(4):
                unit_v(0, si)()

            for j in range(4):
                if j + 1 < 4:
                    dma_x(j + 1)
                ni = 4 * (j + 1)
                # filler queues with emission-order deadlines:
                #  early: v(j,*) during h0 iters < 4j (v[4j+si] consumed at
                #         h0 iter 4j+si; for j=0 it was done in the prologue)
                #  mid:   q/k(j,m1) (consumed from h2) + proj(j-1), over h0-h1
                #  late:  q/k(j+1,m0) (consumed at next j's h0), over h2-h3
                early = [unit_v(j, si) for si in range(4)] if j > 0 else []
                mid = [unit_qk("q", j, 1), unit_qk("k", j, 1)]
                late = ([unit_qk("q", j + 1, 0), unit_qk("k", j + 1, 0)]
                        if j + 1 < 4 else [])
                if j > 0:
                    # pj has no deadline - for the last j keep it late so the
                    # tail drain overlaps attention, else feed it to h0/h1
                    if j == 3:
                        late += [unit_pj(j - 1, 0), unit_pj(j - 1, 1)]
                    else:
                        mid += [unit_pj(j - 1, 0), unit_pj(j - 1, 1)]
                for h in range(HL):
                    if h == 2:  # deadline: h2 reads qT/kT[m=1]
                        while mid:
                            mid.pop(0)()
                    m, po = h // 2, 64 * (h % 2)
                    qh = qT_sb[m][po:po + 64, :]
                    kh = kT_sb[m][po:po + 64, :]
                    ot = ps.tile([128, 512], F32, tag="ot", bufs=2, name=f"ot{j}{h}")

                    if h < 2:
                        fill, stride = (early + mid, max(1, (2 * ni) //
                                        (len(early) + len(mid) + 1)))
                        early, mid = [], []
                    else:
                        fill, stride = late, max(1, (2 * ni) // (len(late) + 1))
                        late = []

                    # software-pipelined emission, lookahead 2: the in-order PE
                    # runs ST(i+1), ST(i+2) while ACT computes exp(i), so OT(i)
                    # rarely stalls and ACT never starves.
                    def emit_st(i):
                        qs = max(512 * j, 128 * i)
                        n = 512 * j + 512 - qs
                        wide = n == 128  # widen: fp32r free-dim<256 is 1/4 rate
                        if wide:
                            qs -= 128
                            n = 256
                        st = ps.tile([128, 512], F32, tag="st", bufs=4,
                                     name=f"st{j}{h}{i}")
                        nc.tensor.matmul(st[:, 0:n], kh[:, 128 * i:128 * i + 128],
                                         qh[:, qs:qs + n], start=True, stop=True)
                        if wide:  # cols 0:128 invalid, 128:256 triangle
                            nc.vector.tensor_tensor(st[:, 0:256], st[:, 0:256],
                                                    maskw_sb, op=OP.add)
                        elif 128 * i >= 512 * j:  # diagonal 128x128 triangle
                            nc.vector.tensor_tensor(st[:, 0:128], st[:, 0:128],
                                                    mask_sb, op=OP.add)
                        pt = ptp.tile([128, 512], F32R, tag="pt", name=f"pt{j}{h}{i}")
                        nc.scalar.activation(pt[:, 0:n], st[:, 0:n], AF.Exp,
                                             scale=0.125)
                        return pt, qs, n

                    def emit_ot(i, pt, qs, n):
                        nc.tensor.matmul(ot[:, qs - 512 * j:512],
                                         vO_sb[i][:, 128 * h:128 * h + 128],
                                         pt[:, 0:n],
                                         start=(i == 0), stop=(i == ni - 1))

                    LOOK = 3
                    pts = {}
                    for i in range(min(LOOK, ni)):
                        pts[i] = emit_st(i)
                    for i in range(ni):
                        if i + LOOK < ni:
                            pts[i + LOOK] = emit_st(i + LOOK)
                        emit_ot(i, *pts.pop(i))
                        # v(j,si) must be emitted before h0 reaches iter 4j+si;
                        # popping one filler per early iteration satisfies it
                        if fill and (i % stride == 0 or
                                     (h == 0 and j > 0 and len(fill) > 4 * j - i)):
                            fill.pop(0)()
                    while fill and h in (1, 3):
                        fill.pop(0)()
                    # normalize: ot rows 64:128 hold broadcast denominators
                    # (exp and ln share an ACT table set - no table reload)
                    # reciprocal on DVE (has slack) instead of Ln+Exp on ACT:
                    # ACT is the saturated engine during attention stretches
                    rb = rcp.tile([64, 512], F32, tag=f"rb{h}", name=f"rb{j}{h}")
                    nc.vector.reciprocal(rb, ot[64:128, :])
                    nc.vector.tensor_tensor(
                        yT_sb[m][po:po + 64, 512 * j:512 * j + 512],
                        ot[0:64, :], rb, op=OP.mult)
            unit_pj(3, 0)()
            unit_pj(3, 1)()

    _install_legalizer(nc)
    return nc


_NC_CACHE = None


def _get_nc():
    global _NC_CACHE
    if _NC_CACHE is None:
        _NC_CACHE = build_nc()
    return _NC_CACHE


def make_in_maps(x, Wq, bq, Wk, Wv, Wp):
    x = np.asarray(x, np.float32)
    xT = [_round_fp32r(np.ascontiguousarray(x[b].T)) for b in range(2)]
    tri = np.where(np.arange(128)[None, :] >= np.arange(128)[:, None],
                   np.float32(0.0), np.float32(MASKVAL)).astype(np.float32)
    in_maps = []
    for c in range(8):
        b, g = c // 4, c % 4
        sl = slice(DL * g, DL * g + DL)
        in_maps.append({
            "xT": xT[b],
            "wq": _round_fp32r(np.asarray(Wq)[sl, :].T),
            "wk": _round_fp32r(np.asarray(Wk)[sl, :].T),
            "wv": _round_fp32r(np.asarray(Wv)[sl, :].T),
            "wp": _round_fp32r(np.asarray(Wp)[:, sl].T),
            "bq": np.ascontiguousarray(np.asarray(bq, np.float32)[sl]),
            "mask": tri,
        })
    return in_maps


def kernel(x, Wq, bq, Wk, bk, Wv, bv, Wp, bp, _run_kwargs=None):
    nc = _get_nc()
    in_maps = make_in_maps(x, Wq, bq, Wk, Wv, Wp)
    res = run_bass_kernel_spmd(nc, in_maps, list(range(8)), **(_run_kwargs or {}))
    corr = (np.asarray(bv, np.float32) @ np.asarray(Wp, np.float32).T
            + np.asarray(bp, np.float32))
    out = np.zeros((2, S, D), np.float32)
    for c in range(8):
        out[c // 4] += res.results[c]["out"]
    out += corr[None, None, :]
    kernel.last_results = res
    return out
